# revision 12
# baseline (speedup 1.0000x reference)
"""Causal self-attention (B=2, T=2048, D=1024, H=16) on 8 trn2 NeuronCores.

Sharding: core c handles batch c//4 and heads 4*(c%4) .. 4*(c%4)+3.
Each core is fully independent (tensor-parallel on heads + data-parallel on
batch); the host does the scatter (slice weights / transpose x) and the
gather (sum partial projection outputs, add bias).

On-chip dataflow (everything in [feature, token] "transposed" layout so no
on-chip transposes are ever needed):
  stage1a: Qt/Kt = (Wq/Wk slice) @ xT      [dims, tok]   f32r matmuls
  stage1b: V     = xT.T @ WvT              [tok, dims]   (x tiles stationary)
  stage2 : St[kv,q] = Kt.T-slices @ Qt     bf16, 2 heads row-packed (K=64)
  softmax: P = exp(St) (ACT -> bf16), causal zeroing via gpsimd
           affine_select, denominator fused into PV as a 65th stationary
           column of ones
  stage3 : ctxT[d,q] (+denom row) = [V|1].T @ P          bf16
  norm   : recip (DVE) -> gpsimd partition_broadcast -> ctx * bcast (DVE)
  stage4 : outT_partial = WprojT-slice.T @ ctxn          f32r -> DMA out

Precision: stage1/4 matmuls run float32r (fp32 storage, single PE pass over
the bf16-high halves); the attention core runs bf16 operands with fp32 PSUM
accumulation — identical operand rounding to f32r, but row-packing works.
f32r matmul operands must be *produced* rounded: DMA'd inputs are
pre-rounded on the host, on-chip producers declare f32r/bf16 output dtypes.
"""

import sys
import numpy as np

for _p in ("/opt/trn_rl_repo", "/opt/pypackages"):
    if _p not in sys.path:
        sys.path.append(_p)

B, T, D, H, HD = 2, 2048, 1024, 16, 64
NCORES = 8
CW = 512   # q-chunk width
KTS = 128  # kv tile size

_CACHE = {}


def _build_nc(t=T):
    from contextlib import ExitStack

    import concourse.bacc as bacc
    import concourse.mybir as mybir
    import concourse.tile as tile

    F32 = mybir.dt.float32
    F32R = mybir.dt.float32r
    BF16 = mybir.dt.bfloat16
    AF = mybir.ActivationFunctionType

    nch = t // CW   # q chunks
    nkv = t // KTS  # kv tiles
    KPC = CW // KTS  # kv tiles per chunk (4)

    nc = bacc.Bacc("TRN2", debug=False)
    xt_d = nc.dram_tensor("xt", [D, t], F32R, kind="ExternalInput")
    wq_d = nc.dram_tensor("wq", [D, 768], F32R, kind="ExternalInput")
    bqk_d = nc.dram_tensor("bqk", [128, 4], F32, kind="ExternalInput")
    bv_d = nc.dram_tensor("bv", [1, 256], F32R, kind="ExternalInput")
    wpj_d = nc.dram_tensor("wpj", [256, D], F32R, kind="ExternalInput")
    out_d = nc.dram_tensor("out", [D, t], F32, kind="ExternalOutput")

    with tile.TileContext(nc) as tc, ExitStack() as ctx:
        res = ctx.enter_context(tc.tile_pool(name="res", bufs=1))
        pst = ctx.enter_context(tc.tile_pool(name="pst", bufs=4, space="PSUM"))
        psc = ctx.enter_context(tc.tile_pool(name="psc", bufs=4, space="PSUM"))
        sbt = ctx.enter_context(tc.tile_pool(name="sbt", bufs=4))
        sbo = ctx.enter_context(tc.tile_pool(name="sbo", bufs=4))

        # ---- resident inputs ----
        # wq split into column blocks so the first stage1a matmuls (block 0)
        # can start after ~0.5MB instead of the full 3MB; DMAs ordered by
        # first use: wq block0, x chunk0, rest of wq, x chunk1, wpj, x rest.
        wq_t = [res.tile([128, 768], F32R, name=f"wq{k}") for k in range(8)]
        xt_t = {(k, ch): res.tile([128, CW], F32R, name=f"xt{k}_{ch}")
                for ch in range(nch) for k in range(8)}

        def dma_wq(mb):
            for k in range(8):
                nc.sync.dma_start(
                    wq_t[k][:, mb * 128:(mb + 1) * 128],
                    wq_d.ap()[k * 128:(k + 1) * 128, mb * 128:(mb + 1) * 128])

        def dma_xt(ch):
            for k in range(8):
                nc.sync.dma_start(
                    xt_t[(k, ch)],
                    xt_d.ap()[k * 128:(k + 1) * 128, ch * CW:(ch + 1) * CW])

        dma_wq(0)
        dma_xt(0)
        for mb in range(1, 6):
            dma_wq(mb)
        bqk_t = res.tile([128, 4], F32, name="bqk")
        nc.sync.dma_start(bqk_t, bqk_d.ap())
        bv_t = res.tile([1, 256], F32R, name="bv")
        nc.sync.dma_start(bv_t, bv_d.ap())
        if nch > 1:
            dma_xt(1)
        wpj_t = []
        for pg in range(2):
            w = res.tile([128, D], F32R, name=f"wpj{pg}")
            nc.sync.dma_start(w, wpj_d.ap()[pg * 128:(pg + 1) * 128, :])
            wpj_t.append(w)
        for ch in range(2, nch):
            dma_xt(ch)

        # memset cannot write f32r; stage in f32 and DVE-copy (casts+rounds)
        ones_f32 = res.tile([1, 128], F32, name="ones_f32")
        nc.vector.memset(ones_f32, 1.0)
        ones_row = res.tile([1, 128], F32R, name="ones_row")
        nc.vector.tensor_copy(ones_row, ones_f32)
        onescol2 = res.tile([128, 2], F32, name="onescol2")
        nc.vector.memset(onescol2, 1.0)

        # ---- resident intermediates ----
        QT = [res.tile([128, t], BF16, name=f"QT{pg}") for pg in range(2)]
        KT = [res.tile([128, t], BF16, name=f"KT{pg}") for pg in range(2)]
        # Vones[pg][kt]: [V_h0 (64) | ones | V_h1 (64) | ones]
        VO = [[res.tile([128, 130], BF16, name=f"VO{pg}_{kt}") for kt in range(nkv)]
              for pg in range(2)]

        ctxn = {}  # (ch, pg) -> normalized ctxT chunk [128, CW]

        def stage1(ch):
            cs = slice(ch * CW, (ch + 1) * CW)
            # stage1a: QT / KT (transposed layout, bf16 out)
            for m in range(4):
                acc = pst.tile([128, CW], F32, name="acc", tag="pt")
                for k in range(8):
                    nc.tensor.matmul(
                        acc,
                        wq_t[k][:, m * 128:(m + 1) * 128],
                        xt_t[(k, ch)],
                        start=(k == 0), stop=(k == 7))
                dest = (QT if m < 2 else KT)[m % 2]
                nc.vector.tensor_scalar_add(dest[:, cs], acc, bqk_t[:, m:m + 1])
            # stage1b: V natural layout
            for sub in range(KPC):
                kt = ch * KPC + sub
                accv = pst.tile([128, 256], F32, name="accv", tag="pt")
                for k in range(8):
                    nc.tensor.matmul(
                        accv,
                        xt_t[(k, ch)][:, sub * 128:(sub + 1) * 128],
                        wq_t[k][:, 512:768],
                        start=(k == 0), stop=False)
                nc.tensor.matmul(
                    accv, ones_row, bv_t, start=False, stop=True)
                for pg in range(2):
                    vo = VO[pg][kt]
                    vo3 = vo.rearrange("p (h c) -> p h c", h=2)
                    nc.vector.tensor_copy(
                        vo3[:, :, 64:65],
                        onescol2.rearrange("p (h c) -> p h c", h=2))
                    src = accv[:, pg * 128:(pg + 1) * 128].rearrange(
                        "p (h c) -> p h c", h=2)
                    nc.vector.tensor_copy(vo3[:, :, 0:64], src)

        def attention(ch, pg):
            cs = slice(ch * CW, (ch + 1) * CW)
            nkt = (ch + 1) * KPC
            cps = [psc.tile([65, CW], F32, name=f"cps{h}", tag="ctx")
                   for h in range(2)]

            def scores(kt):
                # diagonal tiles: columns [0, off) are entirely masked and
                # skipped in the matmul, exp, and PV (ragged accumulation).
                off = max(0, kt * 128 - ch * CW)
                ps = []
                for h in range(2):
                    hs = slice(h * 64, (h + 1) * 64)
                    sps = pst.tile([128, CW], F32, name="sps", tag="pt")
                    nc.tensor.matmul(
                        sps[:, off:],
                        KT[pg][hs, kt * 128:(kt + 1) * 128],
                        QT[pg][hs, cs][:, off:],
                        start=True, stop=True)
                    ps.append(sps)
                pb = []
                for h in range(2):
                    p_sb = sbt.tile([128, CW], BF16, name="p_sb")
                    nc.scalar.activation(p_sb[:, off:], ps[h][:, off:], AF.Exp)
                    if kt >= ch * KPC:
                        # zero the kv > q part of the diagonal block
                        nc.gpsimd.affine_select(
                            out=p_sb[:, off:], in_=p_sb[:, off:],
                            compare_op=mybir.AluOpType.is_ge,
                            fill=0.0,
                            base=0,
                            pattern=[[1, CW - off]],
                            channel_multiplier=-1)
                    pb.append(p_sb)
                return pb

            def pv(kt, pb):
                off = max(0, kt * 128 - ch * CW)
                for h in range(2):
                    nc.tensor.matmul(
                        cps[h][:, off:],
                        VO[pg][kt][:, h * 65:(h + 1) * 65],
                        pb[h][:, off:],
                        start=(kt == 0), stop=(kt == nkt - 1))

            # software pipeline: scores(kt+1) is emitted before pv(kt) so the
            # PE never sits waiting on the exp of the tile it just produced.
            prev = scores(0)
            for kt in range(1, nkt):
                cur = scores(kt)
                pv(kt - 1, prev)
                prev = cur
            pv(nkt - 1, prev)

            # normalize: all off the PE
            cn = sbo.tile([128, CW], F32R, name="cn", tag="cn")
            for h in range(2):
                hs = slice(h * 64, (h + 1) * 64)
                rc = sbt.tile([1, CW], F32, name="rc")
                nc.vector.reciprocal(rc, cps[h][64:65, :])
                bsb = sbt.tile([64, CW], F32, name="bsb")
                nc.gpsimd.partition_broadcast(bsb, rc)
                nc.vector.tensor_mul(cn[hs, :], cps[h][0:64, :], bsb)
            ctxn[(ch, pg)] = cn

        def stage4(ch):
            cs = slice(ch * CW, (ch + 1) * CW)
            for m in range(8):
                ops = pst.tile([128, CW], F32, name="ops", tag="pt")
                for pg in range(2):
                    nc.tensor.matmul(
                        ops,
                        wpj_t[pg][:, m * 128:(m + 1) * 128],
                        ctxn[(ch, pg)],
                        start=(pg == 0), stop=(pg == 1))
                osb = sbo.tile([128, CW], F32, name="osb", tag="osb")
                nc.vector.tensor_copy(osb, ops)
                nc.sync.dma_start(out_d.ap()[m * 128:(m + 1) * 128, cs], osb)

        # software pipeline: stage4(ch) is emitted after stage1(ch+1) so the
        # PE never stalls on the normalize chain of chunk ch.
        stage1(0)
        for ch in range(nch):
            attention(ch, 0)
            attention(ch, 1)
            if ch + 1 < nch:
                stage1(ch + 1)
            stage4(ch)

    nc.compile()
    return nc


def get_nc(t=T):
    key = ("nc", t)
    if key not in _CACHE:
        _CACHE[key] = _build_nc(t)
    return _CACHE[key]


def _round_bf16(a):
    """Round fp32 -> bf16 precision (keep fp32 container, zero low mantissa).

    fp32r matmuls use the high halves of the fp32 words; pre-rounding on the
    host makes the hardware truncation exact.
    """
    import ml_dtypes
    return np.asarray(a, np.float32).astype(ml_dtypes.bfloat16).astype(np.float32)


def make_in_maps(x, W_qkv, b_qkv, W_proj):
    """Host-side scatter: per-core input dict."""
    scale = float(HD) ** -0.5
    x = np.ascontiguousarray(np.asarray(x, np.float32))
    W_qkv = np.asarray(W_qkv, np.float32)
    b_qkv = np.asarray(b_qkv, np.float32)
    W_proj = np.asarray(W_proj, np.float32)
    in_maps = []
    xtb = [_round_bf16(x[b].T) for b in range(B)]
    for c in range(NCORES):
        b = c // 4
        h0 = 4 * (c % 4)          # first head of this core
        r = slice(h0 * 64, h0 * 64 + 256)
        qr = W_qkv[0 * D:1 * D][r] * scale   # [256, D]
        kr = W_qkv[1 * D:2 * D][r]
        vr = W_qkv[2 * D:3 * D][r]
        wq = np.concatenate(
            [qr[:128].T, qr[128:].T, kr[:128].T, kr[128:].T, vr.T], axis=1)
        bqk = np.stack(
            [b_qkv[0 * D:1 * D][r][:128] * scale,
             b_qkv[0 * D:1 * D][r][128:] * scale,
             b_qkv[1 * D:2 * D][r][:128],
             b_qkv[1 * D:2 * D][r][128:]], axis=1)
        bv = b_qkv[2 * D:3 * D][r][None, :]
        wpj = W_proj[:, r].T
        in_maps.append({
            "xt": np.ascontiguousarray(xtb[b]),
            "wq": np.ascontiguousarray(_round_bf16(wq)),
            "bqk": np.ascontiguousarray(bqk),
            "bv": np.ascontiguousarray(_round_bf16(bv)),
            "wpj": np.ascontiguousarray(_round_bf16(wpj)),
        })
    return in_maps


def gather(parts, b_proj):
    """Host-side gather: sum per-core partials, transpose, add bias."""
    b_proj = np.asarray(b_proj, np.float32)
    outs = []
    for b in range(B):
        acc = parts[4 * b]
        for c in range(4 * b + 1, 4 * b + 4):
            acc = acc + parts[c]
        outs.append(acc.T + b_proj)
    return np.stack(outs).astype(np.float32)


def kernel(x, W_qkv, b_qkv, W_proj, b_proj):
    from concourse.bass_utils import run_bass_kernel_spmd

    nc = get_nc()
    in_maps = make_in_maps(x, W_qkv, b_qkv, W_proj)
    import os
    trace = bool(int(os.environ.get("KERNEL_TRACE", "0")))
    res = run_bass_kernel_spmd(nc, in_maps, core_ids=list(range(NCORES)),
                               trace=trace)
    _CACHE["last_results"] = res
    parts = [res.results[c]["out"] for c in range(NCORES)]
    return gather(parts, b_proj)


# revision 14
# speedup vs baseline: 1.1377x; 1.1377x over previous
"""Causal self-attention (B=2, T=2048, D=1024, H=16) on 8 trn2 NeuronCores.

Sharding: core c handles batch c//4 and heads 4*(c%4) .. 4*(c%4)+3.
Each core is fully independent (tensor-parallel on heads + data-parallel on
batch); the host does the scatter (slice weights / transpose x) and the
gather (sum partial projection outputs, add bias).

On-chip dataflow (everything in [feature, token] "transposed" layout so no
on-chip transposes are ever needed):
  stage1a: Qt/Kt = (Wq/Wk slice) @ xT      [dims, tok]   f32r matmuls
  stage1b: V     = xT.T @ WvT              [tok, dims]   (x tiles stationary)
  stage2 : St[kv,q] = Kt.T-slices @ Qt     bf16, 2 heads row-packed (K=64)
  softmax: P = exp(St) (ACT -> bf16), causal zeroing via gpsimd
           affine_select, denominator fused into PV as a 65th stationary
           column of ones
  stage3 : ctxT[d,q] (+denom row) = [V|1].T @ P          bf16
  norm   : recip (DVE) -> gpsimd partition_broadcast -> ctx * bcast (DVE)
  stage4 : outT_partial = WprojT-slice.T @ ctxn          f32r -> DMA out

Precision: stage1/4 matmuls run float32r (fp32 storage, single PE pass over
the bf16-high halves); the attention core runs bf16 operands with fp32 PSUM
accumulation — identical operand rounding to f32r, but row-packing works.
f32r matmul operands must be *produced* rounded: DMA'd inputs are
pre-rounded on the host, on-chip producers declare f32r/bf16 output dtypes.
"""

import sys
import numpy as np

for _p in ("/opt/trn_rl_repo", "/opt/pypackages"):
    if _p not in sys.path:
        sys.path.append(_p)

B, T, D, H, HD = 2, 2048, 1024, 16, 64
NCORES = 8
CW = 512   # q-chunk width
KTS = 128  # kv tile size

_CACHE = {}


def _build_nc(t=T):
    from contextlib import ExitStack

    import concourse.bacc as bacc
    import concourse.mybir as mybir
    import concourse.tile as tile

    F32 = mybir.dt.float32
    F32R = mybir.dt.float32r
    BF16 = mybir.dt.bfloat16
    AF = mybir.ActivationFunctionType

    nch = t // CW   # q chunks
    nkv = t // KTS  # kv tiles
    KPC = CW // KTS  # kv tiles per chunk (4)

    nc = bacc.Bacc("TRN2", debug=False)
    xt_d = nc.dram_tensor("xt", [D, t], F32R, kind="ExternalInput")
    wq_d = nc.dram_tensor("wq", [D, 768], F32R, kind="ExternalInput")
    bqk_d = nc.dram_tensor("bqk", [128, 4], F32, kind="ExternalInput")
    bv_d = nc.dram_tensor("bv", [1, 256], F32R, kind="ExternalInput")
    wpj_d = nc.dram_tensor("wpj", [256, D], F32R, kind="ExternalInput")
    out_d = nc.dram_tensor("out", [D, t], F32, kind="ExternalOutput")

    with tile.TileContext(nc) as tc, ExitStack() as ctx:
        res = ctx.enter_context(tc.tile_pool(name="res", bufs=1))
        pst = ctx.enter_context(tc.tile_pool(name="pst", bufs=4, space="PSUM"))
        psc = ctx.enter_context(tc.tile_pool(name="psc", bufs=4, space="PSUM"))
        sbt = ctx.enter_context(tc.tile_pool(name="sbt", bufs=4))
        sbo = ctx.enter_context(tc.tile_pool(name="sbo", bufs=4))

        # ---- resident inputs ----
        # wq split into column blocks so the first stage1a matmuls (block 0)
        # can start after ~0.5MB instead of the full 3MB; DMAs ordered by
        # first use: wq block0, x chunk0, rest of wq, x chunk1, wpj, x rest.
        wq_t = [res.tile([128, 768], F32R, name=f"wq{k}") for k in range(8)]
        xt_t = {(k, ch): res.tile([128, CW], F32R, name=f"xt{k}_{ch}")
                for ch in range(nch) for k in range(8)}

        def dma_wq(mb):
            for k in range(8):
                nc.sync.dma_start(
                    wq_t[k][:, mb * 128:(mb + 1) * 128],
                    wq_d.ap()[k * 128:(k + 1) * 128, mb * 128:(mb + 1) * 128])

        def dma_xt(ch):
            for k in range(8):
                nc.sync.dma_start(
                    xt_t[(k, ch)],
                    xt_d.ap()[k * 128:(k + 1) * 128, ch * CW:(ch + 1) * CW])

        dma_wq(0)
        dma_xt(0)
        for mb in range(1, 6):
            dma_wq(mb)
        bqk_t = res.tile([128, 4], F32, name="bqk")
        nc.sync.dma_start(bqk_t, bqk_d.ap())
        bv_t = res.tile([1, 256], F32R, name="bv")
        nc.sync.dma_start(bv_t, bv_d.ap())
        if nch > 1:
            dma_xt(1)
        wpj_t = []
        for pg in range(2):
            w = res.tile([128, D], F32R, name=f"wpj{pg}")
            nc.sync.dma_start(w, wpj_d.ap()[pg * 128:(pg + 1) * 128, :])
            wpj_t.append(w)
        for ch in range(2, nch):
            dma_xt(ch)

        # memset cannot write f32r; stage in f32 and DVE-copy (casts+rounds)
        ones_f32 = res.tile([1, 128], F32, name="ones_f32")
        nc.vector.memset(ones_f32, 1.0)
        ones_row = res.tile([1, 128], F32R, name="ones_row")
        nc.vector.tensor_copy(ones_row, ones_f32)
        onescol2 = res.tile([128, 2], F32, name="onescol2")
        nc.vector.memset(onescol2, 1.0)

        # ---- resident intermediates ----
        QT = [res.tile([128, t], BF16, name=f"QT{pg}") for pg in range(2)]
        KT = [res.tile([128, t], BF16, name=f"KT{pg}") for pg in range(2)]
        # Vones[pg][kt]: [V_h0 (64) | ones | V_h1 (64) | ones]
        VO = [[res.tile([128, 130], BF16, name=f"VO{pg}_{kt}") for kt in range(nkv)]
              for pg in range(2)]

        ctxn = {}  # (ch, pg) -> normalized ctxT chunk [128, CW]

        def stage1(ch):
            cs = slice(ch * CW, (ch + 1) * CW)
            # stage1a: QT / KT (transposed layout, bf16 out)
            for m in range(4):
                acc = pst.tile([128, CW], F32, name="acc", tag="pt")
                for k in range(8):
                    nc.tensor.matmul(
                        acc,
                        wq_t[k][:, m * 128:(m + 1) * 128],
                        xt_t[(k, ch)],
                        start=(k == 0), stop=(k == 7))
                dest = (QT if m < 2 else KT)[m % 2]
                nc.vector.tensor_scalar_add(dest[:, cs], acc, bqk_t[:, m:m + 1])
            # stage1b: V natural layout
            for sub in range(KPC):
                kt = ch * KPC + sub
                accv = pst.tile([128, 256], F32, name="accv", tag="pt")
                for k in range(8):
                    nc.tensor.matmul(
                        accv,
                        xt_t[(k, ch)][:, sub * 128:(sub + 1) * 128],
                        wq_t[k][:, 512:768],
                        start=(k == 0), stop=False)
                nc.tensor.matmul(
                    accv, ones_row, bv_t, start=False, stop=True)
                for pg in range(2):
                    vo = VO[pg][kt]
                    vo3 = vo.rearrange("p (h c) -> p h c", h=2)
                    nc.vector.tensor_copy(
                        vo3[:, :, 64:65],
                        onescol2.rearrange("p (h c) -> p h c", h=2))
                    src = accv[:, pg * 128:(pg + 1) * 128].rearrange(
                        "p (h c) -> p h c", h=2)
                    nc.vector.tensor_copy(vo3[:, :, 0:64], src)

        def attention(ch, pg):
            cs = slice(ch * CW, (ch + 1) * CW)
            nkt = (ch + 1) * KPC
            cps = [psc.tile([65, CW], F32, name=f"cps{h}", tag="ctx")
                   for h in range(2)]

            def scores(kt):
                # diagonal tiles: columns [0, off) are entirely masked and
                # skipped in the matmul, exp, and PV (ragged accumulation).
                off = max(0, kt * 128 - ch * CW)
                ps = []
                for h in range(2):
                    hs = slice(h * 64, (h + 1) * 64)
                    sps = pst.tile([128, CW], F32, name="sps", tag="pt")
                    nc.tensor.matmul(
                        sps[:, off:],
                        KT[pg][hs, kt * 128:(kt + 1) * 128],
                        QT[pg][hs, cs][:, off:],
                        start=True, stop=True)
                    ps.append(sps)
                pb = []
                for h in range(2):
                    p_sb = sbt.tile([128, CW], BF16, name="p_sb")
                    nc.scalar.activation(p_sb[:, off:], ps[h][:, off:], AF.Exp)
                    if kt >= ch * KPC:
                        # zero the kv > q part of the diagonal block
                        nc.gpsimd.affine_select(
                            out=p_sb[:, off:], in_=p_sb[:, off:],
                            compare_op=mybir.AluOpType.is_ge,
                            fill=0.0,
                            base=0,
                            pattern=[[1, CW - off]],
                            channel_multiplier=-1)
                    pb.append(p_sb)
                return pb

            def pv(kt, pb):
                off = max(0, kt * 128 - ch * CW)
                for h in range(2):
                    nc.tensor.matmul(
                        cps[h][:, off:],
                        VO[pg][kt][:, h * 65:(h + 1) * 65],
                        pb[h][:, off:],
                        start=(kt == 0), stop=(kt == nkt - 1))

            # software pipeline: scores(kt+1) is emitted before pv(kt) so the
            # PE never sits waiting on the exp of the tile it just produced.
            prev = scores(0)
            for kt in range(1, nkt):
                cur = scores(kt)
                pv(kt - 1, prev)
                prev = cur
            pv(nkt - 1, prev)

            # normalize: all off the PE
            cn = sbo.tile([128, CW], F32R, name="cn", tag="cn")
            for h in range(2):
                hs = slice(h * 64, (h + 1) * 64)
                rc = sbt.tile([1, CW], F32, name="rc")
                den = sbt.tile([1, CW], F32, name="den")
                nc.vector.tensor_copy(den, cps[h][64:65, :])
                # ~5x faster than reciprocal(); 18 bits is plenty for the
                # softmax denominator (inputs are strictly positive, normal)
                nc.vector.reciprocal_approx_fast(rc, den)
                bsb = sbt.tile([64, CW], F32, name="bsb")
                nc.gpsimd.partition_broadcast(bsb, rc)
                nc.vector.tensor_mul(cn[hs, :], cps[h][0:64, :], bsb)
            ctxn[(ch, pg)] = cn

        def stage4(ch):
            cs = slice(ch * CW, (ch + 1) * CW)
            for m in range(8):
                ops = pst.tile([128, CW], F32, name="ops", tag="pt")
                for pg in range(2):
                    nc.tensor.matmul(
                        ops,
                        wpj_t[pg][:, m * 128:(m + 1) * 128],
                        ctxn[(ch, pg)],
                        start=(pg == 0), stop=(pg == 1))
                osb = sbo.tile([128, CW], F32, name="osb", tag="osb")
                nc.vector.tensor_copy(osb, ops)
                nc.sync.dma_start(out_d.ap()[m * 128:(m + 1) * 128, cs], osb)

        # software pipeline: stage4(ch) is emitted after stage1(ch+1) so the
        # PE never stalls on the normalize chain of chunk ch.
        stage1(0)
        for ch in range(nch):
            attention(ch, 0)
            attention(ch, 1)
            if ch + 1 < nch:
                stage1(ch + 1)
            stage4(ch)

    nc.compile()
    return nc


def get_nc(t=T):
    key = ("nc", t)
    if key not in _CACHE:
        _CACHE[key] = _build_nc(t)
    return _CACHE[key]


def _round_bf16(a):
    """Round fp32 -> bf16 precision (keep fp32 container, zero low mantissa).

    fp32r matmuls use the high halves of the fp32 words; pre-rounding on the
    host makes the hardware truncation exact.
    """
    import ml_dtypes
    return np.asarray(a, np.float32).astype(ml_dtypes.bfloat16).astype(np.float32)


def make_in_maps(x, W_qkv, b_qkv, W_proj):
    """Host-side scatter: per-core input dict."""
    scale = float(HD) ** -0.5
    x = np.ascontiguousarray(np.asarray(x, np.float32))
    W_qkv = np.asarray(W_qkv, np.float32)
    b_qkv = np.asarray(b_qkv, np.float32)
    W_proj = np.asarray(W_proj, np.float32)
    in_maps = []
    xtb = [_round_bf16(x[b].T) for b in range(B)]
    for c in range(NCORES):
        b = c // 4
        h0 = 4 * (c % 4)          # first head of this core
        r = slice(h0 * 64, h0 * 64 + 256)
        qr = W_qkv[0 * D:1 * D][r] * scale   # [256, D]
        kr = W_qkv[1 * D:2 * D][r]
        vr = W_qkv[2 * D:3 * D][r]
        wq = np.concatenate(
            [qr[:128].T, qr[128:].T, kr[:128].T, kr[128:].T, vr.T], axis=1)
        bqk = np.stack(
            [b_qkv[0 * D:1 * D][r][:128] * scale,
             b_qkv[0 * D:1 * D][r][128:] * scale,
             b_qkv[1 * D:2 * D][r][:128],
             b_qkv[1 * D:2 * D][r][128:]], axis=1)
        bv = b_qkv[2 * D:3 * D][r][None, :]
        wpj = W_proj[:, r].T
        in_maps.append({
            "xt": np.ascontiguousarray(xtb[b]),
            "wq": np.ascontiguousarray(_round_bf16(wq)),
            "bqk": np.ascontiguousarray(bqk),
            "bv": np.ascontiguousarray(_round_bf16(bv)),
            "wpj": np.ascontiguousarray(_round_bf16(wpj)),
        })
    return in_maps


def gather(parts, b_proj):
    """Host-side gather: sum per-core partials, transpose, add bias."""
    b_proj = np.asarray(b_proj, np.float32)
    outs = []
    for b in range(B):
        acc = parts[4 * b]
        for c in range(4 * b + 1, 4 * b + 4):
            acc = acc + parts[c]
        outs.append(acc.T + b_proj)
    return np.stack(outs).astype(np.float32)


def kernel(x, W_qkv, b_qkv, W_proj, b_proj):
    from concourse.bass_utils import run_bass_kernel_spmd

    nc = get_nc()
    in_maps = make_in_maps(x, W_qkv, b_qkv, W_proj)
    import os
    trace = bool(int(os.environ.get("KERNEL_TRACE", "0")))
    res = run_bass_kernel_spmd(nc, in_maps, core_ids=list(range(NCORES)),
                               trace=trace)
    _CACHE["last_results"] = res
    parts = [res.results[c]["out"] for c in range(NCORES)]
    return gather(parts, b_proj)


# revision 16
# speedup vs baseline: 1.2554x; 1.1034x over previous
"""Causal self-attention (B=2, T=2048, D=1024, H=16) on 8 trn2 NeuronCores.

Sharding: core c handles batch c//4 and heads 4*(c%4) .. 4*(c%4)+3.
Each core is fully independent (tensor-parallel on heads + data-parallel on
batch); the host does the scatter (slice weights / transpose x) and the
gather (sum partial projection outputs, add bias).

On-chip dataflow (everything in [feature, token] "transposed" layout so no
on-chip transposes are ever needed):
  stage1a: Qt/Kt = (Wq/Wk slice) @ xT      [dims, tok]   f32r matmuls
  stage1b: V     = xT.T @ WvT              [tok, dims]   (x tiles stationary)
  stage2 : St[kv,q] = Kt.T-slices @ Qt     bf16, 2 heads row-packed (K=64)
  softmax: P = exp(St) (ACT -> bf16), causal zeroing via gpsimd
           affine_select, denominator fused into PV as a 65th stationary
           column of ones
  stage3 : ctxT[d,q] (+denom row) = [V|1].T @ P          bf16
  norm   : recip (DVE) -> gpsimd partition_broadcast -> ctx * bcast (DVE)
  stage4 : outT_partial = WprojT-slice.T @ ctxn          f32r -> DMA out

Precision: stage1/4 matmuls run float32r (fp32 storage, single PE pass over
the bf16-high halves); the attention core runs bf16 operands with fp32 PSUM
accumulation — identical operand rounding to f32r, but row-packing works.
f32r matmul operands must be *produced* rounded: DMA'd inputs are
pre-rounded on the host, on-chip producers declare f32r/bf16 output dtypes.
"""

import sys
import numpy as np

for _p in ("/opt/trn_rl_repo", "/opt/pypackages"):
    if _p not in sys.path:
        sys.path.append(_p)

B, T, D, H, HD = 2, 2048, 1024, 16, 64
NCORES = 8
CW = 512   # q-chunk width
KTS = 128  # kv tile size

_CACHE = {}


def _build_nc(t=T):
    from contextlib import ExitStack

    import concourse.bacc as bacc
    import concourse.mybir as mybir
    import concourse.tile as tile

    F32 = mybir.dt.float32
    F32R = mybir.dt.float32r
    BF16 = mybir.dt.bfloat16
    AF = mybir.ActivationFunctionType

    nch = t // CW   # q chunks
    nkv = t // KTS  # kv tiles
    KPC = CW // KTS  # kv tiles per chunk (4)

    nc = bacc.Bacc("TRN2", debug=False)
    xt_d = nc.dram_tensor("xt", [D, t], F32R, kind="ExternalInput")
    wq_d = nc.dram_tensor("wq", [D, 768], F32R, kind="ExternalInput")
    bqk_d = nc.dram_tensor("bqk", [128, 4], F32, kind="ExternalInput")
    bv_d = nc.dram_tensor("bv", [1, 256], F32R, kind="ExternalInput")
    wpj_d = nc.dram_tensor("wpj", [256, D], F32R, kind="ExternalInput")
    out_d = nc.dram_tensor("out", [D, t], F32, kind="ExternalOutput")

    with tile.TileContext(nc) as tc, ExitStack() as ctx:
        res = ctx.enter_context(tc.tile_pool(name="res", bufs=1))
        pst = ctx.enter_context(tc.tile_pool(name="pst", bufs=4, space="PSUM"))
        psc = ctx.enter_context(tc.tile_pool(name="psc", bufs=4, space="PSUM"))
        sbt = ctx.enter_context(tc.tile_pool(name="sbt", bufs=4))
        sbo = ctx.enter_context(tc.tile_pool(name="sbo", bufs=4))

        # ---- resident inputs ----
        # DMA issue costs ~0.6us of serial sequencer time per dma_start, so
        # use few, large DMAs: 8 for wq, 8 full-width tiles for x.
        wq_t = []
        for k in range(8):
            w = res.tile([128, 768], F32R, name=f"wq{k}")
            nc.sync.dma_start(w, wq_d.ap()[k * 128:(k + 1) * 128, :])
            wq_t.append(w)
        xt_t = []
        for k in range(8):
            x = res.tile([128, t], F32R, name=f"xt{k}")
            nc.sync.dma_start(x, xt_d.ap()[k * 128:(k + 1) * 128, :])
            xt_t.append(x)
        bqk_t = res.tile([128, 4], F32, name="bqk")
        nc.sync.dma_start(bqk_t, bqk_d.ap())
        bv_t = res.tile([1, 256], F32R, name="bv")
        nc.sync.dma_start(bv_t, bv_d.ap())
        wpj_t = []
        for pg in range(2):
            w = res.tile([128, D], F32R, name=f"wpj{pg}")
            nc.sync.dma_start(w, wpj_d.ap()[pg * 128:(pg + 1) * 128, :])
            wpj_t.append(w)

        # memset cannot write f32r; stage in f32 and DVE-copy (casts+rounds)
        ones_f32 = res.tile([1, 128], F32, name="ones_f32")
        nc.vector.memset(ones_f32, 1.0)
        ones_row = res.tile([1, 128], F32R, name="ones_row")
        nc.vector.tensor_copy(ones_row, ones_f32)
        onescol2 = res.tile([128, 2], F32, name="onescol2")
        nc.vector.memset(onescol2, 1.0)

        # ---- resident intermediates ----
        QT = [res.tile([128, t], BF16, name=f"QT{pg}") for pg in range(2)]
        KT = [res.tile([128, t], BF16, name=f"KT{pg}") for pg in range(2)]
        # Vones[pg][kt]: [V_h0 (64) | ones | V_h1 (64) | ones]
        VO = [[res.tile([128, 130], BF16, name=f"VO{pg}_{kt}") for kt in range(nkv)]
              for pg in range(2)]

        ctxn = {}  # (ch, pg) -> normalized ctxT chunk [128, CW]

        def stage1(ch):
            cs = slice(ch * CW, (ch + 1) * CW)
            # stage1a: QT / KT (transposed layout, bf16 out)
            for m in range(4):
                acc = pst.tile([128, CW], F32, name="acc", tag="pt")
                for k in range(8):
                    nc.tensor.matmul(
                        acc,
                        wq_t[k][:, m * 128:(m + 1) * 128],
                        xt_t[k][:, cs],
                        start=(k == 0), stop=(k == 7))
                dest = (QT if m < 2 else KT)[m % 2]
                nc.vector.tensor_scalar_add(dest[:, cs], acc, bqk_t[:, m:m + 1])
            # stage1b: V natural layout
            for sub in range(KPC):
                kt = ch * KPC + sub
                accv = pst.tile([128, 256], F32, name="accv", tag="pt")
                for k in range(8):
                    nc.tensor.matmul(
                        accv,
                        xt_t[k][:, kt * 128:(kt + 1) * 128],
                        wq_t[k][:, 512:768],
                        start=(k == 0), stop=False)
                nc.tensor.matmul(
                    accv, ones_row, bv_t, start=False, stop=True)
                for pg in range(2):
                    vo = VO[pg][kt]
                    vo3 = vo.rearrange("p (h c) -> p h c", h=2)
                    nc.vector.tensor_copy(
                        vo3[:, :, 64:65],
                        onescol2.rearrange("p (h c) -> p h c", h=2))
                    src = accv[:, pg * 128:(pg + 1) * 128].rearrange(
                        "p (h c) -> p h c", h=2)
                    nc.vector.tensor_copy(vo3[:, :, 0:64], src)

        def attention(ch, pg):
            cs = slice(ch * CW, (ch + 1) * CW)
            nkt = (ch + 1) * KPC
            cps = [psc.tile([65, CW], F32, name=f"cps{h}", tag="ctx")
                   for h in range(2)]

            def scores(kt):
                # diagonal tiles: columns [0, off) are entirely masked and
                # skipped in the matmul, exp, and PV (ragged accumulation).
                off = max(0, kt * 128 - ch * CW)
                ps = []
                for h in range(2):
                    hs = slice(h * 64, (h + 1) * 64)
                    sps = pst.tile([128, CW], F32, name="sps", tag="pt")
                    nc.tensor.matmul(
                        sps[:, off:],
                        KT[pg][hs, kt * 128:(kt + 1) * 128],
                        QT[pg][hs, cs][:, off:],
                        start=True, stop=True)
                    ps.append(sps)
                pb = []
                for h in range(2):
                    p_sb = sbt.tile([128, CW], BF16, name="p_sb")
                    nc.scalar.activation(p_sb[:, off:], ps[h][:, off:], AF.Exp)
                    if kt >= ch * KPC:
                        # zero the kv > q part of the diagonal block
                        nc.gpsimd.affine_select(
                            out=p_sb[:, off:], in_=p_sb[:, off:],
                            compare_op=mybir.AluOpType.is_ge,
                            fill=0.0,
                            base=0,
                            pattern=[[1, CW - off]],
                            channel_multiplier=-1)
                    pb.append(p_sb)
                return pb

            def pv(kt, pb):
                off = max(0, kt * 128 - ch * CW)
                for h in range(2):
                    nc.tensor.matmul(
                        cps[h][:, off:],
                        VO[pg][kt][:, h * 65:(h + 1) * 65],
                        pb[h][:, off:],
                        start=(kt == 0), stop=(kt == nkt - 1))

            # software pipeline: scores(kt+1) is emitted before pv(kt) so the
            # PE never sits waiting on the exp of the tile it just produced.
            prev = scores(0)
            for kt in range(1, nkt):
                cur = scores(kt)
                pv(kt - 1, prev)
                prev = cur
            pv(nkt - 1, prev)

            # normalize: all off the PE
            cn = sbo.tile([128, CW], F32R, name="cn", tag="cn")
            for h in range(2):
                hs = slice(h * 64, (h + 1) * 64)
                rc = sbt.tile([1, CW], F32, name="rc")
                den = sbt.tile([1, CW], F32, name="den")
                nc.vector.tensor_copy(den, cps[h][64:65, :])
                # ~5x faster than reciprocal(); 18 bits is plenty for the
                # softmax denominator (inputs are strictly positive, normal)
                nc.vector.reciprocal_approx_fast(rc, den)
                bsb = sbt.tile([64, CW], F32, name="bsb")
                nc.gpsimd.partition_broadcast(bsb, rc)
                nc.vector.tensor_mul(cn[hs, :], cps[h][0:64, :], bsb)
            ctxn[(ch, pg)] = cn

        def stage4(ch):
            cs = slice(ch * CW, (ch + 1) * CW)
            for m in range(8):
                ops = pst.tile([128, CW], F32, name="ops", tag="pt")
                for pg in range(2):
                    nc.tensor.matmul(
                        ops,
                        wpj_t[pg][:, m * 128:(m + 1) * 128],
                        ctxn[(ch, pg)],
                        start=(pg == 0), stop=(pg == 1))
                osb = sbo.tile([128, CW], F32, name="osb", tag="osb")
                nc.vector.tensor_copy(osb, ops)
                nc.sync.dma_start(out_d.ap()[m * 128:(m + 1) * 128, cs], osb)

        # software pipeline: stage4(ch) is emitted after stage1(ch+1) so the
        # PE never stalls on the normalize chain of chunk ch.
        stage1(0)
        for ch in range(nch):
            attention(ch, 0)
            attention(ch, 1)
            if ch + 1 < nch:
                stage1(ch + 1)
            stage4(ch)

    nc.compile()
    return nc


def get_nc(t=T):
    key = ("nc", t)
    if key not in _CACHE:
        _CACHE[key] = _build_nc(t)
    return _CACHE[key]


def _round_bf16(a):
    """Round fp32 -> bf16 precision (keep fp32 container, zero low mantissa).

    fp32r matmuls use the high halves of the fp32 words; pre-rounding on the
    host makes the hardware truncation exact.
    """
    import ml_dtypes
    return np.asarray(a, np.float32).astype(ml_dtypes.bfloat16).astype(np.float32)


def make_in_maps(x, W_qkv, b_qkv, W_proj):
    """Host-side scatter: per-core input dict."""
    scale = float(HD) ** -0.5
    x = np.ascontiguousarray(np.asarray(x, np.float32))
    W_qkv = np.asarray(W_qkv, np.float32)
    b_qkv = np.asarray(b_qkv, np.float32)
    W_proj = np.asarray(W_proj, np.float32)
    in_maps = []
    xtb = [_round_bf16(x[b].T) for b in range(B)]
    for c in range(NCORES):
        b = c // 4
        h0 = 4 * (c % 4)          # first head of this core
        r = slice(h0 * 64, h0 * 64 + 256)
        qr = W_qkv[0 * D:1 * D][r] * scale   # [256, D]
        kr = W_qkv[1 * D:2 * D][r]
        vr = W_qkv[2 * D:3 * D][r]
        wq = np.concatenate(
            [qr[:128].T, qr[128:].T, kr[:128].T, kr[128:].T, vr.T], axis=1)
        bqk = np.stack(
            [b_qkv[0 * D:1 * D][r][:128] * scale,
             b_qkv[0 * D:1 * D][r][128:] * scale,
             b_qkv[1 * D:2 * D][r][:128],
             b_qkv[1 * D:2 * D][r][128:]], axis=1)
        bv = b_qkv[2 * D:3 * D][r][None, :]
        wpj = W_proj[:, r].T
        in_maps.append({
            "xt": np.ascontiguousarray(xtb[b]),
            "wq": np.ascontiguousarray(_round_bf16(wq)),
            "bqk": np.ascontiguousarray(bqk),
            "bv": np.ascontiguousarray(_round_bf16(bv)),
            "wpj": np.ascontiguousarray(_round_bf16(wpj)),
        })
    return in_maps


def gather(parts, b_proj):
    """Host-side gather: sum per-core partials, transpose, add bias."""
    b_proj = np.asarray(b_proj, np.float32)
    outs = []
    for b in range(B):
        acc = parts[4 * b]
        for c in range(4 * b + 1, 4 * b + 4):
            acc = acc + parts[c]
        outs.append(acc.T + b_proj)
    return np.stack(outs).astype(np.float32)


def kernel(x, W_qkv, b_qkv, W_proj, b_proj):
    from concourse.bass_utils import run_bass_kernel_spmd

    nc = get_nc()
    in_maps = make_in_maps(x, W_qkv, b_qkv, W_proj)
    import os
    trace = bool(int(os.environ.get("KERNEL_TRACE", "0")))
    res = run_bass_kernel_spmd(nc, in_maps, core_ids=list(range(NCORES)),
                               trace=trace)
    _CACHE["last_results"] = res
    parts = [res.results[c]["out"] for c in range(NCORES)]
    return gather(parts, b_proj)
